# revision 2
# baseline (speedup 1.0000x reference)
"""Vocab-parallel dense layer (x @ mat^T + bias) on 8 TRN2 NeuronCores.

Full-input contract: kernel(x, mat, bias) takes the unsharded numpy inputs
  x    (4096, 1, 1024) f32
  mat  (1, 32000, 1024) f32
  bias (1, 32000) f32
and returns the full (4096, 32000) f32 output.

Sharding: mat/bias are split over num_classes into 8 shards of 4000 columns
(tensor/vocab parallel); x is replicated.  Each core computes its
(4096, 4000) output slice with a fully SBUF-resident bf16 matmul; the host
concatenates the slices.
"""

import numpy as np

import concourse.bass as bass
import concourse.bacc as bacc
import concourse.tile as tile
from concourse import mybir
from concourse.bass_utils import run_bass_kernel_spmd

# Problem geometry (hardcoded; harness runs this file standalone).
B = 4096          # batch
E = 1024          # embed size (contraction dim K)
C = 32000         # num classes
NCORES = 8
CS = C // NCORES  # classes per core (4000)

P = 128           # SBUF partitions / matmul tile K and M
KT = E // P       # 8 K-tiles
MT = B // P       # 32 batch tiles
NTILE = 500       # moving free dim per matmul (<=512, fits one PSUM bank in f32)
NT = CS // NTILE  # 8 class tiles per batch tile

_BF16 = mybir.dt.np(mybir.dt.bfloat16)

_program_cache = {}


def _build_program(b=B, e=E, cs=CS, ntile=NTILE, iters=1):
    kt, mt, nt = e // P, b // P, cs // ntile
    bf16, f32 = mybir.dt.bfloat16, mybir.dt.float32

    nc = bacc.Bacc("TRN2", target_bir_lowering=False, debug=False,
                   num_devices=NCORES)
    xT = nc.dram_tensor("xT", (kt, P, b), bf16, kind="ExternalInput").ap()
    matT = nc.dram_tensor("matT", (kt, P, cs), bf16, kind="ExternalInput").ap()
    bias = nc.dram_tensor("bias", (1, cs), f32, kind="ExternalInput").ap()
    out = nc.dram_tensor("out", (b, cs), f32, kind="ExternalOutput").ap()

    with tile.TileContext(nc) as tc:
        with tc.tile_pool(name="resident", bufs=1) as resident, \
             tc.tile_pool(name="psum", bufs=8, space="PSUM") as psums, \
             tc.tile_pool(name="outs", bufs=8) as outs:

            def body():
                # Load x^T, mat^T and broadcast bias; everything stays
                # SBUF-resident for the whole kernel.
                xsb, msb = [], []
                for k in range(kt):
                    xk = resident.tile([P, b], bf16, tag=f"x{k}", name=f"x{k}")
                    nc.sync.dma_start(out=xk[:], in_=xT[k])
                    xsb.append(xk)
                    mk = resident.tile([P, cs], bf16, tag=f"m{k}", name=f"m{k}")
                    nc.sync.dma_start(out=mk[:], in_=matT[k])
                    msb.append(mk)
                bias_sb = resident.tile([P, cs], f32, tag="bias",
                                        name="bias_sb")
                nc.sync.dma_start(out=bias_sb[:],
                                  in_=bias.to_broadcast((P, cs)))

                for m in range(mt):
                    for n in range(nt):
                        ps = psums.tile([P, ntile], f32, tag="ps",
                                        name=f"ps{m}_{n}")
                        for k in range(kt):
                            nc.tensor.matmul(
                                ps[:],
                                xsb[k][:, m * P:(m + 1) * P],
                                msb[k][:, n * ntile:(n + 1) * ntile],
                                start=(k == 0),
                                stop=(k == kt - 1),
                            )
                        ot = outs.tile([P, ntile], f32, tag="ot",
                                       name=f"ot{m}_{n}")
                        nc.vector.tensor_add(
                            out=ot[:], in0=ps[:],
                            in1=bias_sb[:, n * ntile:(n + 1) * ntile])
                        nc.sync.dma_start(
                            out=out[m * P:(m + 1) * P,
                                    n * ntile:(n + 1) * ntile],
                            in_=ot[:])

            if iters == 1:
                body()
            else:
                # Timing variant: run the whole kernel `iters` times on
                # device so wall-clock slope between two iters isolates
                # per-execution device time from dispatch overhead.
                with tc.For_i(0, iters, 1):
                    body()
    nc.compile()
    return nc


def _get_program():
    if "nc" not in _program_cache:
        _program_cache["nc"] = _build_program()
    return _program_cache["nc"]


def _prep_inputs(x, mat, bias):
    """Host-side shard + transpose + bf16 cast."""
    # x: (B, 1, E) -> x^T laid out as (KT, 128, B) bf16, replicated to all cores
    xT = np.ascontiguousarray(x.reshape(B, E).T.astype(_BF16)).reshape(KT, P, B)
    in_maps = []
    m2 = mat.reshape(C, E)
    b2 = bias.reshape(1, C).astype(np.float32)
    for c in range(NCORES):
        shard = m2[c * CS:(c + 1) * CS]  # (CS, E)
        matT = np.ascontiguousarray(shard.T.astype(_BF16)).reshape(KT, P, CS)
        in_maps.append({
            "xT": xT,
            "matT": matT,
            "bias": np.ascontiguousarray(b2[:, c * CS:(c + 1) * CS]),
        })
    return in_maps


def _run(in_maps, trace=False):
    nc = _get_program()
    return run_bass_kernel_spmd(nc, in_maps, core_ids=list(range(NCORES)),
                                trace=trace)


def kernel(x, mat, bias):
    res = _run(_prep_inputs(np.asarray(x), np.asarray(mat), np.asarray(bias)))
    return np.concatenate([res.results[c]["out"] for c in range(NCORES)],
                          axis=1)
